# revision 29
# baseline (speedup 1.0000x reference)
"""Trainium2 Bass kernel for nn_ALNet (adaptive linear network forward).

Math: vals = x @ W + b  ([B,256] @ [256,128] + [128]), then a 7-level
alternating min/max pairwise tree over the 128 leaf columns -> [B, 1].

Strategy (8 NeuronCores, data-parallel over batch):
  - Host: transpose each core's batch shard to xT [256, 8192] (bf16) so the
    contraction dim lands on SBUF partitions with zero on-device transposes;
    bit-reverse-permute W's columns (and b) so the alternating min/max tree
    becomes 7 contiguous half-vs-half tensor_tensor ops.
  - Device per core, per group of 2048 batch rows:
      PE:  bias seeded via rank-1 ones x bias_row matmuls (start=True),
           then x @ W accumulated as LDW(x tile)+MM(W k-tile) pairs in bf16
           (all K-half-0 MMs emitted before K-half-1 so PE starts as soon
           as the first half-DMA lands).
      ACT: evicts PSUM f32 -> SBUF bf16 (copy).
      DVE: 7 halving min/max levels batched across 16 batch-tiles.
  - Output staged as [128, 64] f32 (out[p, c] = batch row 128*c+p),
    de-interleaved on host.
"""

import numpy as np

try:
    import concourse.bass as bass
except ImportError:  # pragma: no cover
    import sys

    sys.path.insert(0, "/opt/trn_rl_repo")
    import concourse.bass as bass

import ml_dtypes
import concourse.mybir as mybir
import concourse.tile as tile
from concourse import bacc
from concourse.bass_utils import run_bass_kernel_spmd

F32 = mybir.dt.float32
BF16 = mybir.dt.bfloat16
F16 = mybir.dt.float16

B, F, NL = 65536, 256, 128
NCORES = 8
BS = B // NCORES  # 8192 batch rows per core

# Tree ops, deepest level first (reference folds reversed root->leaf list;
# the list [min,max,min,...] of length 7 is a palindrome).
_TREE_OPS = [
    mybir.AluOpType.min if i % 2 == 0 else mybir.AluOpType.max for i in range(7)
]


def _bitrev7_perm() -> np.ndarray:
    perm = np.zeros(NL, dtype=np.int64)
    for p in range(NL):
        r = 0
        for k in range(7):
            r |= ((p >> k) & 1) << (6 - k)
        perm[p] = r
    return perm


def build_nc(bs: int = BS, chunk: int = 1024, sup: int = 4096, xbufs: int = 2, xeng: str = 'gpsimd'):
    """bs = batch rows per core; chunk = rows per PSUM group (<=2048,
    mult of 128); sup = rows per x super-load (mult of chunk)."""
    assert chunk % 128 == 0 and bs % chunk == 0
    ncols = bs // 128

    nc = bacc.Bacc(None)
    xT = nc.declare_dram_parameter("xT", [F, bs], F16, isOutput=False)
    Wp = nc.declare_dram_parameter("Wp", [F, NL], F16, isOutput=False)
    brow = nc.declare_dram_parameter("brow", [1, 512], F16, isOutput=False)
    ones = nc.declare_dram_parameter("ones", [1, 128], F16, isOutput=False)
    out = nc.declare_dram_parameter("out", [128, ncols], F32, isOutput=True)

    with tile.TileContext(nc) as tc:
        with (
            tc.tile_pool(name="const", bufs=1) as cpool,
            tc.tile_pool(name="xin", bufs=4) as xpool,
            tc.tile_pool(name="psum", bufs=max(2, (8 * 512) // chunk), space=bass.MemorySpace.PSUM) as ppool,
            tc.tile_pool(name="vals", bufs=3) as vpool,
            tc.tile_pool(name="lvl", bufs=3) as lpool,
            tc.tile_pool(name="ostage", bufs=1) as opool,
        ):
            # constants ride the scalar HWDGE ring so x loads start immediately
            brt = cpool.tile([1, 512], F16, tag="brt")
            ont = cpool.tile([1, 128], F16, tag="ont")
            w0t = cpool.tile([128, NL], F16, tag="w0t")
            w1t = cpool.tile([128, NL], F16, tag="w1t")
            nc.scalar.dma_start(out=brt[:], in_=brow[:])
            nc.scalar.dma_start(out=ont[:], in_=ones[:])
            nc.scalar.dma_start(out=w0t[:], in_=Wp[0:128, :])
            nc.scalar.dma_start(out=w1t[:], in_=Wp[128:256, :])


            ost = opool.tile([128, ncols], F32, tag="ost")

            # tapered super-loads: big first (bandwidth), small last (short tail)
            sups = []
            rem = bs
            plan = [4096, 2048, 1024, 1024]
            for p in plan:
                if rem >= p and p >= chunk:
                    sups.append(p)
                    rem -= p
            while rem:
                p = min(rem, sups[-1] if sups else bs)
                sups.append(p)
                rem -= p
            s0 = 0
            ocol = 0
            for s, sup_s in enumerate(sups):
                gps = sup_s // chunk
                tpb = chunk // 128
                x0 = xpool.tile([128, sup_s], F16, tag="x0", name=f"x0_{s}")
                x1 = xpool.tile([128, sup_s], F16, tag="x1", name=f"x1_{s}")
                nc.sync.dma_start(out=x0[:], in_=xT[0:128, s0 : s0 + sup_s])
                nc.sync.dma_start(out=x1[:], in_=xT[128:256, s0 : s0 + sup_s])
                s0 += sup_s

                pss = [
                    ppool.tile([128, chunk], F32, tag="ps", name=f"ps_{s}_{g}")
                    for g in range(gps)
                ]
                for g in range(gps):
                    for bank in range(chunk // 512):
                        nc.tensor.matmul(
                            pss[g][:, bass.ts(bank, 512)],
                            ont[:],
                            brt[:],
                            start=True,
                            stop=False,
                        )
                for g in range(gps):
                    for t in range(tpb):
                        xsl = bass.ds(g * chunk + t * 128, 128)
                        nc.tensor.matmul(
                            pss[g][:, bass.ts(t, 128)],
                            x0[:, xsl],
                            w0t[:],
                            start=False,
                            stop=False,
                        )
                for g in range(gps):
                    for t in range(tpb):
                        xsl = bass.ds(g * chunk + t * 128, 128)
                        nc.tensor.matmul(
                            pss[g][:, bass.ts(t, 128)],
                            x1[:, xsl],
                            w1t[:],
                            start=False,
                            stop=(t % 4 == 3),
                        )

                tb = min(max(1, 2048 // chunk), gps)
                assert gps % tb == 0
                for gp in range(gps // tb):
                    v = vpool.tile([128, tb * chunk], BF16, tag="v", name=f"v_{s}_{gp}")
                    for q in range(tb):
                        nc.scalar.copy(
                            out=v[:, q * chunk : (q + 1) * chunk],
                            in_=pss[tb * gp + q][:],
                        )
                    nblk = tb * tpb
                    cur = v
                    w = NL // 2
                    for lvl, op in enumerate(_TREE_OPS):
                        r = cur[:].rearrange(
                            "p (blk two h) -> p blk two h", two=2, h=w
                        )
                        in0 = r[:, :, 0, :]
                        in1 = r[:, :, 1, :]
                        if lvl < 6:
                            nxt = lpool.tile(
                                [128, nblk * w], BF16, tag=f"lvl{lvl}", name=f"l{lvl}_{s}_{gp}"
                            )
                            outap = nxt[:].rearrange("p (blk h) -> p blk h", h=w)
                        else:
                            nxt = None
                            outap = ost[:, ocol : ocol + nblk].rearrange(
                                "p (blk h) -> p blk h", h=1
                            )
                            ocol += nblk
                        nc.vector.tensor_tensor(out=outap, in0=in0, in1=in1, op=op)
                        cur = nxt
                        w //= 2

            nc.sync.dma_start(out=out[:], in_=ost[:])

    nc.compile()
    return nc


_NC_CACHE: dict = {}


def _get_nc(bs=BS, chunk=1024, sup=4096, xbufs=4, xeng='sync'):
    key = (bs, chunk, sup, xbufs, xeng)
    if key not in _NC_CACHE:
        _NC_CACHE[key] = build_nc(bs, chunk, sup, xbufs, xeng)
    return _NC_CACHE[key]


def prep_inputs(x: np.ndarray, W: np.ndarray, b: np.ndarray) -> list[dict]:
    perm = _bitrev7_perm()
    bf = ml_dtypes.bfloat16
    Wp = np.ascontiguousarray(W[:, perm]).astype(np.float16)
    bp = np.ascontiguousarray(b[perm]).astype(np.float16)
    brow = np.ascontiguousarray(np.tile(bp[None, :], (1, 4)))  # [1, 512]
    ones = np.ones((1, 128), dtype=np.float16)
    x = np.asarray(x, dtype=np.float32)
    in_maps = []
    for i in range(NCORES):
        xTi = np.ascontiguousarray(x[i * BS : (i + 1) * BS, :].T).astype(np.float16)
        in_maps.append({"xT": xTi, "Wp": Wp, "brow": brow, "ones": ones})
    return in_maps


def gather_outputs(results: list[dict]) -> np.ndarray:
    shards = []
    for i in range(NCORES):
        o = np.asarray(results[i]["out"])  # [128, BS//128]; o[p, c] = row 128c+p
        shards.append(o.T.reshape(BS))
    return np.concatenate(shards).reshape(B, 1).astype(np.float32)


def _setup_tracing():
    """Install the antenv.axon_hooks NTFF-profile shim (missing from this
    image) and neuter the artifact upload so traced runs stay local."""
    import sys as _sys
    import types

    import concourse.bass_utils as bu

    bu.upload_artifacts = lambda tmpdir: tmpdir
    try:
        from antenv.axon_hooks import get_axon_ntff_profile_hook  # noqa: F401

        return
    except ImportError:
        pass
    import antenv

    m = types.ModuleType("antenv.axon_hooks")
    _state = {"hook": None}
    m.set_axon_ntff_profile_hook = lambda h: _state.__setitem__("hook", h)
    m.get_axon_ntff_profile_hook = lambda: _state["hook"]
    _sys.modules["antenv.axon_hooks"] = m
    antenv.axon_hooks = m
    try:
        from trn_agent_boot.trn_boot import _ntff_profile_via_ctypes

        hook = _ntff_profile_via_ctypes("/opt/axon/libaxon_pjrt.so")
        if hook is not None:
            m.set_axon_ntff_profile_hook(hook)
    except Exception as e:  # pragma: no cover
        print("ntff hook install failed:", e)


def run_on_hw(x, W, b, trace: bool = False, **kwargs):
    if trace:
        _setup_tracing()
    nc = _get_nc()
    in_maps = prep_inputs(np.asarray(x), np.asarray(W), np.asarray(b))
    return run_bass_kernel_spmd(
        nc, in_maps, core_ids=list(range(NCORES)), trace=trace, **kwargs
    )


def kernel(x: np.ndarray, W: np.ndarray, b: np.ndarray) -> np.ndarray:
    res = run_on_hw(x, W, b, trace=False)
    return gather_outputs(res.results)


# revision 30
# speedup vs baseline: 1.0048x; 1.0048x over previous
"""Trainium2 Bass kernel for nn_ALNet (adaptive linear network forward).

Math: vals = x @ W + b  ([B,256] @ [256,128] + [128]), then a 7-level
alternating min/max pairwise tree over the 128 leaf columns -> [B, 1].

Strategy (8 NeuronCores, data-parallel over batch):
  - Host: transpose each core's batch shard to xT [256, 8192] (bf16) so the
    contraction dim lands on SBUF partitions with zero on-device transposes;
    bit-reverse-permute W's columns (and b) so the alternating min/max tree
    becomes 7 contiguous half-vs-half tensor_tensor ops.
  - Device per core, per group of 2048 batch rows:
      PE:  bias seeded via rank-1 ones x bias_row matmuls (start=True),
           then x @ W accumulated as LDW(x tile)+MM(W k-tile) pairs in bf16
           (all K-half-0 MMs emitted before K-half-1 so PE starts as soon
           as the first half-DMA lands).
      ACT: evicts PSUM f32 -> SBUF bf16 (copy).
      DVE: 7 halving min/max levels batched across 16 batch-tiles.
  - Output staged as [128, 64] f32 (out[p, c] = batch row 128*c+p),
    de-interleaved on host.
"""

import numpy as np

try:
    import concourse.bass as bass
except ImportError:  # pragma: no cover
    import sys

    sys.path.insert(0, "/opt/trn_rl_repo")
    import concourse.bass as bass

import ml_dtypes
import concourse.mybir as mybir
import concourse.tile as tile
from concourse import bacc
from concourse.bass_utils import run_bass_kernel_spmd

F32 = mybir.dt.float32
BF16 = mybir.dt.bfloat16
F16 = mybir.dt.float16

B, F, NL = 65536, 256, 128
NCORES = 8
BS = B // NCORES  # 8192 batch rows per core

# Tree ops, deepest level first (reference folds reversed root->leaf list;
# the list [min,max,min,...] of length 7 is a palindrome).
_TREE_OPS = [
    mybir.AluOpType.min if i % 2 == 0 else mybir.AluOpType.max for i in range(7)
]


def _bitrev7_perm() -> np.ndarray:
    perm = np.zeros(NL, dtype=np.int64)
    for p in range(NL):
        r = 0
        for k in range(7):
            r |= ((p >> k) & 1) << (6 - k)
        perm[p] = r
    return perm


def build_nc(bs: int = BS, chunk: int = 1024, sup: int = 4096, xbufs: int = 2, xeng: str = 'gpsimd'):
    """bs = batch rows per core; chunk = rows per PSUM group (<=2048,
    mult of 128); sup = rows per x super-load (mult of chunk)."""
    assert chunk % 128 == 0 and bs % chunk == 0
    ncols = bs // 128

    nc = bacc.Bacc(None)
    xT = nc.declare_dram_parameter("xT", [F, bs], F16, isOutput=False)
    Wp = nc.declare_dram_parameter("Wp", [F, NL], F16, isOutput=False)
    brow = nc.declare_dram_parameter("brow", [1, 512], F16, isOutput=False)
    ones = nc.declare_dram_parameter("ones", [1, 128], F16, isOutput=False)
    out = nc.declare_dram_parameter("out", [128, ncols], F32, isOutput=True)

    with tile.TileContext(nc) as tc:
        with (
            tc.tile_pool(name="const", bufs=1) as cpool,
            tc.tile_pool(name="xin", bufs=4) as xpool,
            tc.tile_pool(name="psum", bufs=max(2, (8 * 512) // chunk), space=bass.MemorySpace.PSUM) as ppool,
            tc.tile_pool(name="vals", bufs=2) as vpool,
            tc.tile_pool(name="lvl", bufs=2) as lpool,
            tc.tile_pool(name="ostage", bufs=1) as opool,
        ):
            # constants ride the scalar HWDGE ring so x loads start immediately
            brt = cpool.tile([1, 512], F16, tag="brt")
            ont = cpool.tile([1, 128], F16, tag="ont")
            w0t = cpool.tile([128, NL], F16, tag="w0t")
            w1t = cpool.tile([128, NL], F16, tag="w1t")
            nc.scalar.dma_start(out=brt[:], in_=brow[:])
            nc.scalar.dma_start(out=ont[:], in_=ones[:])
            nc.scalar.dma_start(out=w0t[:], in_=Wp[0:128, :])
            nc.scalar.dma_start(out=w1t[:], in_=Wp[128:256, :])


            ost = opool.tile([128, ncols], F32, tag="ost")

            # tapered super-loads: big first (bandwidth), small last (short tail)
            sups = []
            rem = bs
            plan = [4096, 2048, 1024, 1024]
            for p in plan:
                if rem >= p and p >= chunk:
                    sups.append(p)
                    rem -= p
            while rem:
                p = min(rem, sups[-1] if sups else bs)
                sups.append(p)
                rem -= p
            s0 = 0
            ocol = 0
            for s, sup_s in enumerate(sups):
                gps = sup_s // chunk
                tpb = chunk // 128
                x0 = xpool.tile([128, sup_s], F16, tag="x0", name=f"x0_{s}")
                x1 = xpool.tile([128, sup_s], F16, tag="x1", name=f"x1_{s}")
                nc.sync.dma_start(out=x0[:], in_=xT[0:128, s0 : s0 + sup_s])
                nc.sync.dma_start(out=x1[:], in_=xT[128:256, s0 : s0 + sup_s])
                s0 += sup_s

                pss = [
                    ppool.tile([128, chunk], F32, tag="ps", name=f"ps_{s}_{g}")
                    for g in range(gps)
                ]
                for g in range(gps):
                    for bank in range(chunk // 512):
                        nc.tensor.matmul(
                            pss[g][:, bass.ts(bank, 512)],
                            ont[:],
                            brt[:],
                            start=True,
                            stop=False,
                        )
                for g in range(gps):
                    for t in range(tpb):
                        xsl = bass.ds(g * chunk + t * 128, 128)
                        nc.tensor.matmul(
                            pss[g][:, bass.ts(t, 128)],
                            x0[:, xsl],
                            w0t[:],
                            start=False,
                            stop=False,
                        )
                for g in range(gps):
                    for t in range(tpb):
                        xsl = bass.ds(g * chunk + t * 128, 128)
                        nc.tensor.matmul(
                            pss[g][:, bass.ts(t, 128)],
                            x1[:, xsl],
                            w1t[:],
                            start=False,
                            stop=(t % 4 == 3),
                        )

                tb = min(max(1, 2048 // chunk), gps)
                assert gps % tb == 0
                for gp in range(gps // tb):
                    v = vpool.tile([128, tb * chunk], BF16, tag="v", name=f"v_{s}_{gp}")
                    for q in range(tb):
                        nc.scalar.copy(
                            out=v[:, q * chunk : (q + 1) * chunk],
                            in_=pss[tb * gp + q][:],
                        )
                    nblk = tb * tpb
                    cur = v
                    w = NL // 2
                    for lvl, op in enumerate(_TREE_OPS):
                        r = cur[:].rearrange(
                            "p (blk two h) -> p blk two h", two=2, h=w
                        )
                        in0 = r[:, :, 0, :]
                        in1 = r[:, :, 1, :]
                        if lvl < 6:
                            nxt = lpool.tile(
                                [128, nblk * w], BF16, tag=f"lvl{lvl}", name=f"l{lvl}_{s}_{gp}"
                            )
                            outap = nxt[:].rearrange("p (blk h) -> p blk h", h=w)
                        else:
                            nxt = None
                            outap = ost[:, ocol : ocol + nblk].rearrange(
                                "p (blk h) -> p blk h", h=1
                            )
                            ocol += nblk
                        nc.vector.tensor_tensor(out=outap, in0=in0, in1=in1, op=op)
                        cur = nxt
                        w //= 2

            nc.sync.dma_start(out=out[:], in_=ost[:])

    nc.compile()
    return nc


_NC_CACHE: dict = {}


def _get_nc(bs=BS, chunk=1024, sup=4096, xbufs=4, xeng='sync'):
    key = (bs, chunk, sup, xbufs, xeng)
    if key not in _NC_CACHE:
        _NC_CACHE[key] = build_nc(bs, chunk, sup, xbufs, xeng)
    return _NC_CACHE[key]


def prep_inputs(x: np.ndarray, W: np.ndarray, b: np.ndarray) -> list[dict]:
    perm = _bitrev7_perm()
    bf = ml_dtypes.bfloat16
    Wp = np.ascontiguousarray(W[:, perm]).astype(np.float16)
    bp = np.ascontiguousarray(b[perm]).astype(np.float16)
    brow = np.ascontiguousarray(np.tile(bp[None, :], (1, 4)))  # [1, 512]
    ones = np.ones((1, 128), dtype=np.float16)
    x = np.asarray(x, dtype=np.float32)
    in_maps = []
    for i in range(NCORES):
        xTi = np.ascontiguousarray(x[i * BS : (i + 1) * BS, :].T).astype(np.float16)
        in_maps.append({"xT": xTi, "Wp": Wp, "brow": brow, "ones": ones})
    return in_maps


def gather_outputs(results: list[dict]) -> np.ndarray:
    shards = []
    for i in range(NCORES):
        o = np.asarray(results[i]["out"])  # [128, BS//128]; o[p, c] = row 128c+p
        shards.append(o.T.reshape(BS))
    return np.concatenate(shards).reshape(B, 1).astype(np.float32)


def _setup_tracing():
    """Install the antenv.axon_hooks NTFF-profile shim (missing from this
    image) and neuter the artifact upload so traced runs stay local."""
    import sys as _sys
    import types

    import concourse.bass_utils as bu

    bu.upload_artifacts = lambda tmpdir: tmpdir
    try:
        from antenv.axon_hooks import get_axon_ntff_profile_hook  # noqa: F401

        return
    except ImportError:
        pass
    import antenv

    m = types.ModuleType("antenv.axon_hooks")
    _state = {"hook": None}
    m.set_axon_ntff_profile_hook = lambda h: _state.__setitem__("hook", h)
    m.get_axon_ntff_profile_hook = lambda: _state["hook"]
    _sys.modules["antenv.axon_hooks"] = m
    antenv.axon_hooks = m
    try:
        from trn_agent_boot.trn_boot import _ntff_profile_via_ctypes

        hook = _ntff_profile_via_ctypes("/opt/axon/libaxon_pjrt.so")
        if hook is not None:
            m.set_axon_ntff_profile_hook(hook)
    except Exception as e:  # pragma: no cover
        print("ntff hook install failed:", e)


def run_on_hw(x, W, b, trace: bool = False, **kwargs):
    if trace:
        _setup_tracing()
    nc = _get_nc()
    in_maps = prep_inputs(np.asarray(x), np.asarray(W), np.asarray(b))
    return run_bass_kernel_spmd(
        nc, in_maps, core_ids=list(range(NCORES)), trace=trace, **kwargs
    )


def kernel(x: np.ndarray, W: np.ndarray, b: np.ndarray) -> np.ndarray:
    res = run_on_hw(x, W, b, trace=False)
    return gather_outputs(res.results)
